# revision 28
# baseline (speedup 1.0000x reference)
"""Causal single-head attention on 8 Trainium2 NeuronCores.

Shapes (hardcoded per problem spec):
  input_tensor [512, 256, 384] f32, Wq/Wk/Wv [384, 64] f32 -> out [512, 256, 64] f32

Sharding: data-parallel on the batch dim, 64 batches per core, weights
replicated.

Per-group pipeline (GB=2 batches per group; S=256 as two 128-row blocks,
E=384 as three 128-row chunks):
  0. Warmup burst of dummy matmuls covers the x-DMA startup latency and
     flips the PE HAM clock-gate to 2.4 GHz before real work starts
     (PE-transposes don't register as HAM activity, so without this the
     first ~20us of matmuls run at 1.2 GHz).  The x loads for the first
     groups are issued before the weight loads so their descriptor
     generation isn't queued behind 5us of weight DMAs.
  1. DMA x pair [128,2,2,384] into SBUF with f32->f16 cast (SWDGE).
  2. PE-transpose x -> xT, one PSUM half-tile (1 bank) per batch: the
     PSUM->SBUF copy of half b overlaps the transposes of half b+1, so the
     next group's transposes never stall on a PSUM bank.
  3. [kT;vT] = [Wk|Wv].T @ xT -> [128,512]; qT = Wq.T @ xT -> [64,512].
     f16 inputs, f32 PSUM.  PSUM->SBUF casts are spread across engines:
     qT + kv b0-half on ScalarE (ACT copy), kv b1-half on DVE, so the two
     halves cast in parallel and the DVE keeps headroom.
  4. PE-transpose vT back to natural v [256,64] + ones column (col 64
     accumulates the softmax denominator inside the AV matmul).
  5. Scores sT[k,q] = kT.T @ qT per batch: k0 vs all q (N=256), k1 vs q1.
  6. p = exp(0.125*sT) on ScalarE (softmax shift-invariance: scores ~
     N(0,1), no max subtraction needed); causal mask on the two diagonal
     blocks: b0 on GpSimd (affine_select), b1 on DVE (tri multiply).
  7. out_unnorm = p.T @ [v|1] accumulated over causal k blocks only.
  8. One reciprocal + one broadcast multiply per group (f32) on DVE,
     DMA out.
"""

import numpy as np

import concourse.bass as bass
import concourse.mybir as mybir
import concourse.tile as tile
from concourse import bacc
from concourse.bass import ds, ts
from concourse.bass_utils import run_bass_kernel_spmd
from concourse.masks import make_identity, make_upper_triangular

EMBED = 384
HEAD_DIM = 64
SEQ = 256
BATCH = 512
NCORES = 8
NB = BATCH // NCORES  # batches per core

F32 = mybir.dt.float32
F16 = mybir.dt.float16

EC = EMBED // 128  # 3 embed chunks
ST = SEQ // 128    # 2 seq blocks

WARM = 55
PRELOAD = 2  # x loads issued ahead of the weight loads


def _build(nb=NB, warm=WARM):
    """Build the per-core Bass program for nb batches (processed in pairs)."""
    MD = F16
    assert nb % 2 == 0
    GB = 2               # batches per group
    GS = GB * SEQ        # 512: grouped seq columns
    ng = nb // GB

    nc = bacc.Bacc("TRN2", target_bir_lowering=False)
    x = nc.dram_tensor("x", [nb, SEQ, EMBED], F32, kind="ExternalInput")
    wq = nc.dram_tensor("wq", [EMBED, HEAD_DIM], F32, kind="ExternalInput")
    wk = nc.dram_tensor("wk", [EMBED, HEAD_DIM], F32, kind="ExternalInput")
    wv = nc.dram_tensor("wv", [EMBED, HEAD_DIM], F32, kind="ExternalInput")
    out = nc.dram_tensor("out", [nb, SEQ, HEAD_DIM], F32, kind="ExternalOutput")

    xv = x[:, :, :].rearrange("(g b) (t p) e -> g p b t e", b=GB, p=128)
    ov = out[:, :, :].rearrange("(g b) (t p) d -> g p b t d", b=GB, p=128)

    with tile.TileContext(nc) as tc:
        with (
            tc.tile_pool(name="const", bufs=1) as cpool,
            tc.tile_pool(name="sb_x", bufs=4) as sb_x,
            tc.tile_pool(name="sb_xt", bufs=4) as sb_xt,
            tc.tile_pool(name="sb_qk", bufs=4) as sb_qk,
            tc.tile_pool(name="sb_v", bufs=4) as sb_v,
            tc.tile_pool(name="sb_p", bufs=4) as sb_p,
            tc.tile_pool(name="sb_o", bufs=4) as sb_o,
            tc.tile_pool(name="ps_xt0", bufs=1, space="PSUM") as ps_xt0,
            tc.tile_pool(name="ps_xt1", bufs=1, space="PSUM") as ps_xt1,
            tc.tile_pool(name="ps_kv", bufs=1, space="PSUM") as ps_kv,
            tc.tile_pool(name="ps_q", bufs=1, space="PSUM") as ps_q,
            tc.tile_pool(name="ps_vn", bufs=1, space="PSUM") as ps_vn,
            tc.tile_pool(name="ps_st0", bufs=1, space="PSUM") as ps_st0,
            tc.tile_pool(name="ps_st1", bufs=1, space="PSUM") as ps_st1,
            tc.tile_pool(name="ps_av", bufs=1, space="PSUM") as ps_av,
        ):
            ident = cpool.tile([128, 128], MD)
            make_identity(nc, ident)
            # tri[k, q] = 1.0 where k <= q else 0.0
            tri = cpool.tile([128, 128], MD)
            make_upper_triangular(nc, tri, val=1.0, diag=True)
            tri_b = bass.AP(
                tensor=tri.tensor,
                offset=tri.offset,
                ap=[tri.ap[0], [0, 2], [1, 128]],
            )

            # x loads for the first groups, ahead of the weight loads
            xs_pre = []
            for g in range(min(PRELOAD, ng)):
                xs = sb_x.tile([128, GB, ST, EMBED], MD, tag="xs")
                nc.gpsimd.dma_start(out=xs[:, :, :, :], in_=xv[g])
                xs_pre.append(xs)

            # HAM warmup (reuses the av pool ring: no extra PSUM bank)
            if warm:
                warm_ps = ps_av.tile([128, GB, ST, HEAD_DIM + 1], F32, tag="av")
                warm_flat = warm_ps[:, :, :, :].rearrange("p b t w -> p (b t w)")
                for _ in range(warm):
                    nc.tensor.matmul(
                        warm_flat[:, 0:128], ident[:, :], ident[:, :],
                        start=True, stop=True,
                    )

            # [Wk|Wv] packed: kT at parts 0:64, vT at 64:128; f32->f16 in DMA
            wkv_sb = cpool.tile([128, EC, 128], MD)
            nc.gpsimd.dma_start(
                out=wkv_sb[:, :, 0:HEAD_DIM],
                in_=wk[:, :].rearrange("(c p) d -> p c d", p=128),
            )
            nc.gpsimd.dma_start(
                out=wkv_sb[:, :, HEAD_DIM:128],
                in_=wv[:, :].rearrange("(c p) d -> p c d", p=128),
            )
            wq_sb = cpool.tile([128, EC, HEAD_DIM], MD)
            nc.gpsimd.dma_start(
                out=wq_sb[:, :, :],
                in_=wq[:, :].rearrange("(c p) d -> p c d", p=128),
            )

            AW = HEAD_DIM + 1   # 65: v columns + ones column
            for g in range(ng):
                # 1. load a pair of batches with f32 -> f16 cast
                if g < PRELOAD:
                    xs = xs_pre[g]
                else:
                    xs = sb_x.tile([128, GB, ST, EMBED], MD, tag="xs")
                    nc.gpsimd.dma_start(out=xs[:, :, :, :], in_=xv[g])

                # 2. transpose x -> xT; per-batch PSUM half tiles
                xts = sb_xt.tile([128, EC, GS], MD, tag="xts")
                for b, ps_xt in ((0, ps_xt0), (1, ps_xt1)):
                    xt_ps = ps_xt.tile([128, EC * SEQ], MD, tag="xt")
                    for t in range(ST):
                        for c in range(EC):
                            nc.tensor.transpose(
                                xt_ps[:, ds(c * SEQ + t * 128, 128)],
                                xs[:, b, t, ts(c, 128)],
                                ident[:, :],
                            )
                    nc.vector.tensor_copy(
                        xts[:, :, ds(b * SEQ, SEQ)],
                        xt_ps[:, :].rearrange("p (c s) -> p c s", c=EC),
                    )

                # 3. [kT; vT] and qT projections over both batches (N=512)
                kv_ps = ps_kv.tile([128, GS], F32, tag="kv")
                q_ps = ps_q.tile([HEAD_DIM, GS], F32, tag="q")
                for c in range(EC):
                    nc.tensor.matmul(
                        q_ps[:, :], wq_sb[:, c, :], xts[:, c, :],
                        start=(c == 0), stop=(c == EC - 1),
                    )
                for c in range(EC):
                    nc.tensor.matmul(
                        kv_ps[:, :], wkv_sb[:, c, :], xts[:, c, :],
                        start=(c == 0), stop=(c == EC - 1),
                    )
                # casts PSUM->SBUF: qT + kv b0-half on ACT, kv b1-half on
                # DVE - the halves run on different engines in parallel.
                qt_sb = sb_qk.tile([HEAD_DIM, GB, SEQ], MD, tag="qt_sb")
                nc.scalar.copy(
                    qt_sb[:, :, :],
                    q_ps[:, :].rearrange("p (b s) -> p b s", b=GB),
                )
                kv_sb = sb_qk.tile([128, GB, SEQ], MD, tag="kv_sb")
                kv_v = kv_ps[:, :].rearrange("p (b s) -> p b s", b=GB)
                nc.scalar.copy(kv_sb[:, 0, :], kv_v[:, 0, :])
                nc.vector.tensor_copy(kv_sb[:, 1, :], kv_v[:, 1, :])

                # 4. transpose vT back to natural v; ones column appended
                vn_ps = ps_vn.tile([128, GB * ST * HEAD_DIM], MD, tag="vn")
                for b in range(GB):
                    for t in range(ST):
                        nc.tensor.transpose(
                            vn_ps[:, ds((b * ST + t) * HEAD_DIM, HEAD_DIM)],
                            kv_sb[HEAD_DIM:128, b, ts(t, 128)],
                            ident[HEAD_DIM:128, HEAD_DIM:128],
                        )
                v_sb = sb_v.tile([128, GB, ST, AW], MD, tag="v_sb")
                nc.vector.tensor_copy(
                    v_sb[:, :, :, 0:HEAD_DIM],
                    vn_ps[:, :].rearrange("p (b t d) -> p b t d", b=GB, t=ST),
                )
                nc.vector.memset(v_sb[:, :, :, HEAD_DIM:AW], 1.0)

                # 5. scores sT[k, q]: k0 vs all q (N=256), k1 vs q1 (N=128)
                st0 = ps_st0.tile([128, SEQ + 128], F32, tag="st0")
                st1 = ps_st1.tile([128, SEQ + 128], F32, tag="st1")
                for b, stp in ((0, st0), (1, st1)):
                    nc.tensor.matmul(
                        stp[:, 0:SEQ],
                        kv_sb[0:HEAD_DIM, b, 0:128],
                        qt_sb[:, b, :],
                        start=True, stop=True,
                    )
                    nc.tensor.matmul(
                        stp[:, SEQ : SEQ + 128],
                        kv_sb[0:HEAD_DIM, b, 128:256],
                        qt_sb[:, b, 128:256],
                        start=True, stop=True,
                    )

                # 6. p = exp(sT/8) on ACT; causal mask on the two diagonal
                # blocks (cols 0:128, 256:384): b0 on GpSimd, b1 on DVE.
                pt0 = sb_p.tile([128, SEQ + 128], MD, tag="pt0")
                pt1 = sb_p.tile([128, SEQ + 128], MD, tag="pt1")
                nc.scalar.activation(
                    pt0[:, :], st0[:, :],
                    mybir.ActivationFunctionType.Exp, scale=0.125,
                )
                nc.scalar.activation(
                    pt1[:, :], st1[:, :],
                    mybir.ActivationFunctionType.Exp, scale=0.125,
                )
                diag0 = bass.AP(
                    tensor=pt0.tensor, offset=pt0.offset,
                    ap=[pt0.ap[0], [SEQ, 2], [1, 128]],
                )
                # keep p where q >= k (iota = -k + q), zero elsewhere
                nc.gpsimd.affine_select(
                    out=diag0, in_=diag0,
                    pattern=[[0, 2], [1, 128]],
                    compare_op=mybir.AluOpType.is_ge,
                    fill=0.0,
                    base=0, channel_multiplier=-1,
                )
                diag1 = bass.AP(
                    tensor=pt1.tensor, offset=pt1.offset,
                    ap=[pt1.ap[0], [SEQ, 2], [1, 128]],
                )
                nc.vector.tensor_mul(diag1, diag1, tri_b)

                # 7. out_unnorm = p.T @ [v|1]  (col 64 = denominator)
                av_ps = ps_av.tile([128, GB, ST, AW], F32, tag="av")
                for b, pt in ((0, pt0), (1, pt1)):
                    nc.tensor.matmul(
                        av_ps[:, b, 0, :],
                        pt[:, 0:128], v_sb[:, b, 0, :],
                        start=True, stop=True,
                    )
                    nc.tensor.matmul(
                        av_ps[:, b, 1, :],
                        pt[:, 128:256], v_sb[:, b, 0, :],
                        start=True, stop=False,
                    )
                    nc.tensor.matmul(
                        av_ps[:, b, 1, :],
                        pt[:, 256:384], v_sb[:, b, 1, :],
                        start=False, stop=True,
                    )

                # 8. normalize rows (f32): one reciprocal + one broadcast
                # multiply per group on DVE
                out_sb = sb_o.tile([128, GB, ST, HEAD_DIM], F32, tag="out_sb")
                linv = sb_o.tile([128, GB * ST], F32, tag="linv")
                nc.vector.reciprocal(
                    linv[:, :],
                    av_ps[:, :, :, HEAD_DIM : HEAD_DIM + 1].rearrange(
                        "p b t o -> p (b t o)"
                    ),
                )
                # normalize + store per batch: the second half's multiply
                # overlaps the first half's DMA, and the final group's store
                # drains in half-size chunks (shorter kernel tail)
                for b in range(GB):
                    linv_b = bass.AP(
                        tensor=linv.tensor, offset=linv.offset + b * ST,
                        ap=[linv.ap[0], [1, ST], [0, HEAD_DIM]],
                    )
                    nc.vector.tensor_mul(
                        out_sb[:, b, :, :], av_ps[:, b, :, 0:HEAD_DIM], linv_b
                    )
                    nc.sync.dma_start(out=ov[g][:, b], in_=out_sb[:, b, :, :])

    nc.compile()
    return nc


_NC_CACHE = {}


def _get_nc(nb=NB):
    if nb not in _NC_CACHE:
        _NC_CACHE[nb] = _build(nb)
    return _NC_CACHE[nb]


def kernel(input_tensor, Wq, Wk, Wv, **run_kwargs):
    x = np.ascontiguousarray(np.asarray(input_tensor, dtype=np.float32))
    wq = np.ascontiguousarray(np.asarray(Wq, dtype=np.float32))
    wk = np.ascontiguousarray(np.asarray(Wk, dtype=np.float32))
    wv = np.ascontiguousarray(np.asarray(Wv, dtype=np.float32))

    nb = x.shape[0] // NCORES
    nc = _get_nc(nb=nb)
    in_maps = [
        {"x": x[i * nb : (i + 1) * nb], "wq": wq, "wk": wk, "wv": wv}
        for i in range(NCORES)
    ]
    res = run_bass_kernel_spmd(nc, in_maps, core_ids=list(range(NCORES)), **run_kwargs)
    outs = np.concatenate([res.results[i]["out"] for i in range(NCORES)], axis=0)
    if run_kwargs.get("trace"):
        kernel.last_results = res
    return outs


# revision 29
# speedup vs baseline: 1.1594x; 1.1594x over previous
"""Causal single-head attention on 8 Trainium2 NeuronCores.

Shapes (hardcoded per problem spec):
  input_tensor [512, 256, 384] f32, Wq/Wk/Wv [384, 64] f32 -> out [512, 256, 64] f32

Sharding: data-parallel on the batch dim, 64 batches per core, weights
replicated.

Per-group pipeline (GB=2 batches per group; S=256 as two 128-row blocks,
E=384 as three 128-row chunks):
  0. Warmup burst of dummy matmuls covers the x-DMA startup latency and
     flips the PE HAM clock-gate to 2.4 GHz before real work starts
     (PE-transposes don't register as HAM activity, so without this the
     first ~20us of matmuls run at 1.2 GHz).  The x loads for the first
     groups are issued before the weight loads so their descriptor
     generation isn't queued behind 5us of weight DMAs.
  1. DMA x pair [128,2,2,384] into SBUF with f32->f16 cast (SWDGE).
  2. PE-transpose x -> xT, one PSUM half-tile (1 bank) per batch: the
     PSUM->SBUF copy of half b overlaps the transposes of half b+1, so the
     next group's transposes never stall on a PSUM bank.
  3. [kT;vT] = [Wk|Wv].T @ xT -> [128,512]; qT = Wq.T @ xT -> [64,512].
     f16 inputs, f32 PSUM.  PSUM->SBUF casts are spread across engines:
     qT + kv b0-half on ScalarE (ACT copy), kv b1-half on DVE, so the two
     halves cast in parallel and the DVE keeps headroom.
  4. PE-transpose vT back to natural v [256,64] + ones column (col 64
     accumulates the softmax denominator inside the AV matmul).
  5. Scores sT[k,q] = kT.T @ qT per batch: k0 vs all q (N=256), k1 vs q1.
  6. p = exp(0.125*sT) on ScalarE (softmax shift-invariance: scores ~
     N(0,1), no max subtraction needed); causal mask on the two diagonal
     blocks: b0 on GpSimd (affine_select), b1 on DVE (tri multiply).
  7. out_unnorm = p.T @ [v|1] accumulated over causal k blocks only.
  8. One reciprocal + one broadcast multiply per group (f32) on DVE,
     DMA out.
"""

import numpy as np

import concourse.bass as bass
import concourse.mybir as mybir
import concourse.tile as tile
from concourse import bacc
from concourse.bass import ds, ts
from concourse.bass_utils import run_bass_kernel_spmd
from concourse.masks import make_identity, make_upper_triangular

EMBED = 384
HEAD_DIM = 64
SEQ = 256
BATCH = 512
NCORES = 8
NB = BATCH // NCORES  # batches per core

F32 = mybir.dt.float32
F16 = mybir.dt.float16

EC = EMBED // 128  # 3 embed chunks
ST = SEQ // 128    # 2 seq blocks

WARM = 55
PRELOAD = 2  # x loads issued ahead of the weight loads


def _build(nb=NB, warm=WARM):
    """Build the per-core Bass program for nb batches (processed in pairs)."""
    MD = F16
    assert nb % 2 == 0
    GB = 2               # batches per group
    GS = GB * SEQ        # 512: grouped seq columns
    ng = nb // GB

    nc = bacc.Bacc("TRN2", target_bir_lowering=False)
    x = nc.dram_tensor("x", [nb, SEQ, EMBED], F32, kind="ExternalInput")
    wq = nc.dram_tensor("wq", [EMBED, HEAD_DIM], F32, kind="ExternalInput")
    wk = nc.dram_tensor("wk", [EMBED, HEAD_DIM], F32, kind="ExternalInput")
    wv = nc.dram_tensor("wv", [EMBED, HEAD_DIM], F32, kind="ExternalInput")
    out = nc.dram_tensor("out", [nb, SEQ, HEAD_DIM], F32, kind="ExternalOutput")

    xv = x[:, :, :].rearrange("(g b) (t p) e -> g p b t e", b=GB, p=128)
    ov = out[:, :, :].rearrange("(g b) (t p) d -> g p b t d", b=GB, p=128)

    with tile.TileContext(nc) as tc:
        with (
            tc.tile_pool(name="const", bufs=1) as cpool,
            tc.tile_pool(name="sb_x", bufs=4) as sb_x,
            tc.tile_pool(name="sb_xt", bufs=4) as sb_xt,
            tc.tile_pool(name="sb_qk", bufs=4) as sb_qk,
            tc.tile_pool(name="sb_v", bufs=4) as sb_v,
            tc.tile_pool(name="sb_p", bufs=4) as sb_p,
            tc.tile_pool(name="sb_o", bufs=4) as sb_o,
            tc.tile_pool(name="ps_xt0", bufs=1, space="PSUM") as ps_xt0,
            tc.tile_pool(name="ps_xt1", bufs=1, space="PSUM") as ps_xt1,
            tc.tile_pool(name="ps_kv", bufs=1, space="PSUM") as ps_kv,
            tc.tile_pool(name="ps_q", bufs=1, space="PSUM") as ps_q,
            tc.tile_pool(name="ps_vn", bufs=1, space="PSUM") as ps_vn,
            tc.tile_pool(name="ps_st0", bufs=1, space="PSUM") as ps_st0,
            tc.tile_pool(name="ps_st1", bufs=1, space="PSUM") as ps_st1,
            tc.tile_pool(name="ps_av", bufs=1, space="PSUM") as ps_av,
        ):
            ident = cpool.tile([128, 128], MD)
            make_identity(nc, ident)
            # tri[k, q] = 1.0 where k <= q else 0.0
            tri = cpool.tile([128, 128], MD)
            make_upper_triangular(nc, tri, val=1.0, diag=True)
            tri_b = bass.AP(
                tensor=tri.tensor,
                offset=tri.offset,
                ap=[tri.ap[0], [0, 2], [1, 128]],
            )

            # x loads for the first groups, ahead of the weight loads
            xs_pre = []
            for g in range(min(PRELOAD, ng)):
                xs = sb_x.tile([128, GB, ST, EMBED], MD, tag="xs")
                nc.gpsimd.dma_start(out=xs[:, :, :, :], in_=xv[g])
                xs_pre.append(xs)

            # HAM warmup (reuses the av pool ring: no extra PSUM bank)
            if warm:
                warm_ps = ps_av.tile([128, GB, ST, HEAD_DIM + 1], F32, tag="av")
                warm_flat = warm_ps[:, :, :, :].rearrange("p b t w -> p (b t w)")
                for _ in range(warm):
                    nc.tensor.matmul(
                        warm_flat[:, 0:128], ident[:, :], ident[:, :],
                        start=True, stop=True,
                    )

            # [Wk|Wv] packed: kT at parts 0:64, vT at 64:128; f32->f16 in DMA
            wkv_sb = cpool.tile([128, EC, 128], MD)
            nc.gpsimd.dma_start(
                out=wkv_sb[:, :, 0:HEAD_DIM],
                in_=wk[:, :].rearrange("(c p) d -> p c d", p=128),
            )
            nc.gpsimd.dma_start(
                out=wkv_sb[:, :, HEAD_DIM:128],
                in_=wv[:, :].rearrange("(c p) d -> p c d", p=128),
            )
            wq_sb = cpool.tile([128, EC, HEAD_DIM], MD)
            nc.gpsimd.dma_start(
                out=wq_sb[:, :, :],
                in_=wq[:, :].rearrange("(c p) d -> p c d", p=128),
            )

            AW = HEAD_DIM + 1   # 65: v columns + ones column
            for g in range(ng):
                # 1. load a pair of batches with f32 -> f16 cast
                if g < PRELOAD:
                    xs = xs_pre[g]
                else:
                    xs = sb_x.tile([128, GB, ST, EMBED], MD, tag="xs")
                    nc.gpsimd.dma_start(out=xs[:, :, :, :], in_=xv[g])

                # 2. transpose x -> xT; per-batch PSUM half tiles
                xts = sb_xt.tile([128, EC, GS], MD, tag="xts")
                for b, ps_xt in ((0, ps_xt0), (1, ps_xt1)):
                    xt_ps = ps_xt.tile([128, EC * SEQ], MD, tag="xt")
                    for t in range(ST):
                        for c in range(EC):
                            nc.tensor.transpose(
                                xt_ps[:, ds(c * SEQ + t * 128, 128)],
                                xs[:, b, t, ts(c, 128)],
                                ident[:, :],
                            )
                    nc.vector.tensor_copy(
                        xts[:, :, ds(b * SEQ, SEQ)],
                        xt_ps[:, :].rearrange("p (c s) -> p c s", c=EC),
                    )

                # 3. [kT; vT] and qT projections over both batches (N=512)
                kv_ps = ps_kv.tile([128, GS], F32, tag="kv")
                q_ps = ps_q.tile([HEAD_DIM, GS], F32, tag="q")
                for c in range(EC):
                    nc.tensor.matmul(
                        q_ps[:, :], wq_sb[:, c, :], xts[:, c, :],
                        start=(c == 0), stop=(c == EC - 1),
                    )
                for c in range(EC):
                    nc.tensor.matmul(
                        kv_ps[:, :], wkv_sb[:, c, :], xts[:, c, :],
                        start=(c == 0), stop=(c == EC - 1),
                    )
                # casts PSUM->SBUF: qT + kv b0-half on ACT, kv b1-half on
                # DVE - the halves run on different engines in parallel.
                qt_sb = sb_qk.tile([HEAD_DIM, GB, SEQ], MD, tag="qt_sb")
                nc.scalar.copy(
                    qt_sb[:, :, :],
                    q_ps[:, :].rearrange("p (b s) -> p b s", b=GB),
                )
                kv_sb = sb_qk.tile([128, GB, SEQ], MD, tag="kv_sb")
                kv_v = kv_ps[:, :].rearrange("p (b s) -> p b s", b=GB)
                nc.scalar.copy(kv_sb[:, 0, :], kv_v[:, 0, :])
                nc.vector.tensor_copy(kv_sb[:, 1, :], kv_v[:, 1, :])

                # 4. transpose vT back to natural v; ones column appended
                vn_ps = ps_vn.tile([128, GB * ST * HEAD_DIM], MD, tag="vn")
                for b in range(GB):
                    for t in range(ST):
                        nc.tensor.transpose(
                            vn_ps[:, ds((b * ST + t) * HEAD_DIM, HEAD_DIM)],
                            kv_sb[HEAD_DIM:128, b, ts(t, 128)],
                            ident[HEAD_DIM:128, HEAD_DIM:128],
                        )
                v_sb = sb_v.tile([128, GB, ST, AW], MD, tag="v_sb")
                nc.vector.tensor_copy(
                    v_sb[:, :, :, 0:HEAD_DIM],
                    vn_ps[:, :].rearrange("p (b t d) -> p b t d", b=GB, t=ST),
                )
                nc.vector.memset(v_sb[:, :, :, HEAD_DIM:AW], 1.0)

                # 5. scores sT[k, q]: k0 vs all q (N=256), k1 vs q1 (N=128)
                st0 = ps_st0.tile([128, SEQ + 128], F32, tag="st0")
                st1 = ps_st1.tile([128, SEQ + 128], F32, tag="st1")
                for b, stp in ((0, st0), (1, st1)):
                    nc.tensor.matmul(
                        stp[:, 0:SEQ],
                        kv_sb[0:HEAD_DIM, b, 0:128],
                        qt_sb[:, b, :],
                        start=True, stop=True,
                    )
                    nc.tensor.matmul(
                        stp[:, SEQ : SEQ + 128],
                        kv_sb[0:HEAD_DIM, b, 128:256],
                        qt_sb[:, b, 128:256],
                        start=True, stop=True,
                    )

                # 6. p = exp(sT/8) on ACT; causal mask on the two diagonal
                # blocks (cols 0:128, 256:384): b0 on GpSimd, b1 on DVE.
                pt0 = sb_p.tile([128, SEQ + 128], MD, tag="pt0")
                pt1 = sb_p.tile([128, SEQ + 128], MD, tag="pt1")
                nc.scalar.activation(
                    pt0[:, :], st0[:, :],
                    mybir.ActivationFunctionType.Exp, scale=0.125,
                )
                nc.scalar.activation(
                    pt1[:, :], st1[:, :],
                    mybir.ActivationFunctionType.Exp, scale=0.125,
                )
                diag0 = bass.AP(
                    tensor=pt0.tensor, offset=pt0.offset,
                    ap=[pt0.ap[0], [SEQ, 2], [1, 128]],
                )
                # keep p where q >= k (iota = -k + q), zero elsewhere
                nc.gpsimd.affine_select(
                    out=diag0, in_=diag0,
                    pattern=[[0, 2], [1, 128]],
                    compare_op=mybir.AluOpType.is_ge,
                    fill=0.0,
                    base=0, channel_multiplier=-1,
                )
                diag1 = bass.AP(
                    tensor=pt1.tensor, offset=pt1.offset,
                    ap=[pt1.ap[0], [SEQ, 2], [1, 128]],
                )
                nc.vector.tensor_mul(diag1, diag1, tri_b)

                # 7. out_unnorm = p.T @ [v|1]  (col 64 = denominator)
                av_ps = ps_av.tile([128, GB, ST, AW], F32, tag="av")
                for b, pt in ((0, pt0), (1, pt1)):
                    nc.tensor.matmul(
                        av_ps[:, b, 0, :],
                        pt[:, 0:128], v_sb[:, b, 0, :],
                        start=True, stop=True,
                    )
                    nc.tensor.matmul(
                        av_ps[:, b, 1, :],
                        pt[:, 128:256], v_sb[:, b, 0, :],
                        start=True, stop=False,
                    )
                    nc.tensor.matmul(
                        av_ps[:, b, 1, :],
                        pt[:, 256:384], v_sb[:, b, 1, :],
                        start=False, stop=True,
                    )

                # 8. normalize rows (f32): one reciprocal + one broadcast
                # multiply per group on DVE
                out_sb = sb_o.tile([128, GB, ST, HEAD_DIM], F32, tag="out_sb")
                linv = sb_o.tile([128, GB * ST], F32, tag="linv")
                nc.vector.reciprocal(
                    linv[:, :],
                    av_ps[:, :, :, HEAD_DIM : HEAD_DIM + 1].rearrange(
                        "p b t o -> p (b t o)"
                    ),
                )
                linv_b = bass.AP(
                    tensor=linv.tensor, offset=linv.offset,
                    ap=[linv.ap[0], [ST, GB], [1, ST], [0, HEAD_DIM]],
                )
                nc.vector.tensor_mul(
                    out_sb[:, :, :, :], av_ps[:, :, :, 0:HEAD_DIM], linv_b
                )
                nc.sync.dma_start(out=ov[g], in_=out_sb[:, :, :, :])

    nc.compile()
    return nc


_NC_CACHE = {}


def _get_nc(nb=NB):
    if nb not in _NC_CACHE:
        _NC_CACHE[nb] = _build(nb)
    return _NC_CACHE[nb]


def kernel(input_tensor, Wq, Wk, Wv, **run_kwargs):
    x = np.ascontiguousarray(np.asarray(input_tensor, dtype=np.float32))
    wq = np.ascontiguousarray(np.asarray(Wq, dtype=np.float32))
    wk = np.ascontiguousarray(np.asarray(Wk, dtype=np.float32))
    wv = np.ascontiguousarray(np.asarray(Wv, dtype=np.float32))

    nb = x.shape[0] // NCORES
    nc = _get_nc(nb=nb)
    in_maps = [
        {"x": x[i * nb : (i + 1) * nb], "wq": wq, "wk": wk, "wv": wv}
        for i in range(NCORES)
    ]
    res = run_bass_kernel_spmd(nc, in_maps, core_ids=list(range(NCORES)), **run_kwargs)
    outs = np.concatenate([res.results[i]["out"] for i in range(NCORES)], axis=0)
    if run_kwargs.get("trace"):
        kernel.last_results = res
    return outs
